# revision 29
# baseline (speedup 1.0000x reference)
"""EvolutionBank (circular-buffer scatter + gather) on 8 Trainium2 NeuronCores.

Strategy (node-dim sharding, host-routed):
  - Host computes occurrence ranks / slots / per-node counts (tiny int arrays).
  - bank is block-sharded along the node dim: core m owns nodes
    [m*R, (m+1)*R).  The (idx, emb) write tuples and the gather requests are
    routed to the owning core on the host (the "all-to-all by node id" from
    the sharding hint, done as a host-side permutation so no device
    collectives are needed).
  - On each core the Bass kernel does:
      phase 1: scatter routed emb rows into the bank shard at
               (node*W + slot) row granularity (1KB rows) via indirect SWDGE
               DMA (bit-exact copy)
      phase 2: gather (W*D)-sized node rows (8KB) for every routed read with
               the extended-ISA dma_gather (512 rows / 4MB per instruction)
               and stream them to a dense output buffer (pipelined with
               HWDGE stores).
  - times2[idx] and ptr2 are tiny (3.2MB / 1.6MB) and are produced on the
    host; the device handles the ~1.9GB of bank traffic.
"""

import math
from contextlib import ExitStack

import numpy as np

M = 8  # NeuronCores
PART = 128  # SBUF partitions
NB = 2  # gather double-buffers
GCH = 4  # tiles (of 128 rows) per dma_gather instruction
GARBAGE_NODES = 16  # extra node rows; scatter pads land here
SCATTER_MODE = "indirect"  # "indirect" (bit-exact copies) | "delta32" (CCE int32)


def _build_program(
    node_rows: int,
    T: int,
    W: int,
    D: int,
    repeat: int = 1,
    variant: str = "full",
    scatter_mode: str = "indirect",
    NS: int = 0,
    gch: int | None = None,
    nb: int | None = None,
):
    """Bass program shared by all cores.

    node_rows = R + GARBAGE_NODES rows of W*D floats
    T         = number of 128-request tiles (multiple of GCH)
    repeat > 1 re-runs the (idempotent) pipeline for device timing; production
    uses repeat=1.  variant: 'full' | 'gather' | 'scatter' (timing ablations).
    """
    from concourse import bacc, bass, library_config, mybir

    f32 = mybir.dt.float32
    i32 = mybir.dt.int32
    i16 = mybir.dt.int16
    WD = W * D
    do_scatter = variant in ("full", "scatter")
    do_gather = variant in ("full", "gather")
    GCH = gch or globals()["GCH"]
    NB = nb or globals()["NB"]
    assert T % GCH == 0
    NG = T // GCH  # dma_gather instructions per rep
    NI = GCH * PART  # indices per dma_gather
    ICOLS = NI // 16  # int16 idx columns per dma_gather

    delta_mode = scatter_mode == "delta32"
    if delta_mode:
        assert NS > 0 and NS % PART == 0
    NSB = NS // PART if delta_mode else 0  # 128-token blocks per slot
    SCOLS = NS // 16 if delta_mode else 0  # int16 idx columns per slot

    nc = bacc.Bacc("TRN2")
    bankw = nc.declare_dram_parameter("bankw", [node_rows, WD], f32, isOutput=False)
    if delta_mode:
        sdeltat = nc.declare_dram_parameter(
            "sdeltat", [PART, W * NSB, D], i32, isOutput=False
        )
        sidx16 = nc.declare_dram_parameter(
            "sidx16", [PART, W * SCOLS], i16, isOutput=False
        )
    else:
        embt = nc.declare_dram_parameter("embt", [PART, T * D], f32, isOutput=False)
        sidx = nc.declare_dram_parameter("sidx", [PART, T], i32, isOutput=False)
    gidx16 = nc.declare_dram_parameter("gidx16", [PART, NG * ICOLS], i16, isOutput=False)
    outb = nc.declare_dram_parameter("outb", [T * PART, WD], f32, isOutput=True)

    # scatter-side view of the bank: [node_rows*W, D] rows of 1KB
    bank_rows_view = bass.AP(bankw, 0, [[D, node_rows * W], [1, D]])

    ctx = ExitStack()
    with ctx:
        if delta_mode:
            sdelta_s = ctx.enter_context(
                nc.sbuf_tensor("sdelta_s", [PART, W * NSB, D], i32)
            )
            sidx16_s = ctx.enter_context(
                nc.sbuf_tensor("sidx16_s", [PART, W * SCOLS], i16)
            )
        else:
            emb_s = ctx.enter_context(nc.sbuf_tensor("emb_s", [PART, T * D], f32))
            sidx_s = ctx.enter_context(nc.sbuf_tensor("sidx_s", [PART, T], i32))
        gidx16_s = ctx.enter_context(
            nc.sbuf_tensor("gidx16_s", [PART, NG * ICOLS], i16)
        )
        gbufs = [
            ctx.enter_context(nc.sbuf_tensor(f"gbuf{k}", [PART, GCH, WD], f32))
            for k in range(NB)
        ]
        sem_in_a = ctx.enter_context(nc.semaphore("sem_in_a"))
        sem_in_b = ctx.enter_context(nc.semaphore("sem_in_b"))
        sem_sc = [
            ctx.enter_context(nc.semaphore(f"sem_sc{r}")) for r in range(repeat)
        ]
        sga = [ctx.enter_context(nc.semaphore(f"sga{k}")) for k in range(NB)]
        sst = [ctx.enter_context(nc.semaphore(f"sst{k}")) for k in range(NB)]
        block = ctx.enter_context(nc.Block())

        @block.sync
        def _(sync):
            if delta_mode:
                sync.dma_start(out=sidx16_s[:], in_=sidx16[:]).then_inc(sem_in_a, 16)
                sync.dma_start(out=gidx16_s[:], in_=gidx16[:]).then_inc(sem_in_a, 16)
                sync.dma_start(out=sdelta_s[:], in_=sdeltat[:]).then_inc(sem_in_b, 16)
            else:
                sync.dma_start(out=sidx_s[:], in_=sidx[:]).then_inc(sem_in_a, 16)
                sync.dma_start(out=gidx16_s[:], in_=gidx16[:]).then_inc(sem_in_a, 16)
                sync.dma_start(out=emb_s[:], in_=embt[:]).then_inc(sem_in_b, 16)
            for rep in range(repeat):
                if not do_gather:
                    break
                for gi in range(NG):
                    j = rep * NG + gi
                    k, r = j % NB, j // NB
                    sync.wait_ge(sga[k], 16 * (r + 1))
                    for q in range(GCH):
                        # second wait trivially satisfied; same-sem producer order
                        sync.wait_ge(sst[k], 16 * (GCH * r + q))
                        t = gi * GCH + q
                        sync.dma_start(
                            out=outb[t * PART : (t + 1) * PART, :],
                            in_=gbufs[k][:, q, :],
                        ).then_inc(sst[k], 16)

        @block.gpsimd
        def _(gp):
            gp.load_library(library_config.mlp)
            gp.wait_ge(sem_in_a, 32)
            gp.wait_ge(sem_in_b, 16)
            for rep in range(repeat):
                if rep >= 1 and do_gather:
                    # serialize reps: all previous stores done
                    for k in range(NB):
                        cnt = GCH * ((rep * NG - k + NB - 1) // NB)
                        gp.wait_ge(sst[k], 16 * cnt)
                if do_scatter and delta_mode:
                    for s in range(W):
                        out_ap = bass.AP(
                            bankw, s * D, [[WD, node_rows], [1, D]]
                        ).bitcast(i32)
                        gp.dma_scatter_add(
                            out_ap,
                            sdelta_s[:, s * NSB : (s + 1) * NSB, :],
                            sidx16_s[:, s * SCOLS : (s + 1) * SCOLS],
                            NS,
                            NS,
                            D,
                            elem_step=WD,
                        ).then_inc(sem_sc[rep], 16)
                    gp.wait_ge(sem_sc[rep], 16 * W)
                elif do_scatter:
                    for i in range(T):
                        gp.indirect_dma_start(
                            out=bank_rows_view,
                            out_offset=bass.IndirectOffsetOnAxis(
                                ap=sidx_s[:, i : i + 1], axis=0
                            ),
                            in_=emb_s[:, i * D : (i + 1) * D],
                            in_offset=None,
                        ).then_inc(sem_sc[rep], 16)
                    gp.wait_ge(sem_sc[rep], 16 * T)
                if not do_gather:
                    continue
                for gi in range(NG):
                    j = rep * NG + gi
                    k, r = j % NB, j // NB
                    if r >= 1:
                        gp.wait_ge(sst[k], 16 * GCH * r)  # slot free
                        gp.wait_ge(sga[k], 16 * r)  # same-sem producer order
                    gp.dma_gather(
                        gbufs[k][:],
                        bankw[:],
                        gidx16_s[:, gi * ICOLS : (gi + 1) * ICOLS],
                        NI,
                        NI,
                        WD,
                    ).then_inc(sga[k], 16)

    nc.compile()
    return nc


def _host_prep(bank, emb, ptr, idx, n_cores, scatter_mode="indirect"):
    """Compute write slots + route reads/writes to owning cores.

    Returns (per_core_inputs, per_core_sel, T, node_rows, counts, slot, NS).
    """
    N, W, D = bank.shape
    B = idx.shape[0]
    R = math.ceil(N / n_cores)

    idx64 = idx.astype(np.int64)
    # occurrence rank (stable) — matches the reference's sequential semantics
    order = np.argsort(idx64, kind="stable")
    s = idx64[order]
    pos = np.arange(B, dtype=np.int64)
    is_start = np.concatenate([[True], s[1:] != s[:-1]])
    group_start = np.maximum.accumulate(np.where(is_start, pos, 0))
    rank = np.empty(B, np.int64)
    rank[order] = pos - group_start

    slot = (ptr[idx].astype(np.int64) + rank) % W
    counts = np.bincount(idx64, minlength=N).astype(ptr.dtype)

    owner = idx64 // R
    local = idx64 - owner * R

    sels = [np.flatnonzero(owner == m) for m in range(n_cores)]
    bmax = max((len(s_) for s_ in sels), default=1)
    T = max(1, math.ceil(bmax / PART))
    T = ((T + GCH - 1) // GCH) * GCH  # whole dma_gather chunks
    Bpad = T * PART
    node_rows = R + GARBAGE_NODES
    NG = T // GCH
    ICOLS = GCH * PART // 16

    delta_mode = scatter_mode == "delta32"
    NS = 0
    if delta_mode:
        # per-(core, slot) write counts -> common padded token count
        smax = 1
        for m in range(n_cores):
            sl = slot[sels[m]]
            if len(sl):
                smax = max(smax, int(np.bincount(sl, minlength=W).max()))
        NS = ((smax + PART - 1) // PART) * PART

    per_core = []
    for m in range(n_cores):
        sel = sels[m]
        bm = len(sel)

        # bank shard (+ zero pad nodes if N % n_cores != 0, + garbage rows)
        shard = np.zeros((node_rows, W * D), np.float32)
        lo, hi = m * R, min((m + 1) * R, N)
        shard[: hi - lo] = bank[lo:hi].reshape(hi - lo, -1)

        # gather sources: node row index; pads -> node 0
        gflat = np.zeros(Bpad, np.int16)
        gflat[:bm] = local[sel].astype(np.int16)

        # int16 gather indices: request i of chunk gi sits at
        # [i % 16, gi*ICOLS + i//16], replicated down the partition dim
        g16 = (
            gflat.reshape(NG, ICOLS, 16).transpose(2, 0, 1).reshape(16, NG * ICOLS)
        )

        core_in = {
            "bankw": shard,
            "gidx16": np.ascontiguousarray(np.tile(g16, (PART // 16, 1))),
        }

        if delta_mode:
            # int32-bit deltas grouped by slot: device CCE-adds them onto the
            # bank's bit pattern (exact two's-complement reconstruction)
            nodes_m = local[sel]
            slots_m = slot[sel]
            old_bits = shard.reshape(node_rows, W, D)[nodes_m, slots_m].view(np.int32)
            new_bits = emb[sel].view(np.int32)
            with np.errstate(over="ignore"):
                dbits = (new_bits.astype(np.int64) - old_bits.astype(np.int64)).astype(
                    np.int64
                ) & 0xFFFFFFFF
            dbits = dbits.astype(np.uint32).view(np.int32).reshape(bm, D)
            sdelta = np.zeros((W, NS, D), np.int32)
            sidx16v = np.zeros((W, NS), np.int16)
            for s in range(W):
                rows = np.flatnonzero(slots_m == s)
                sdelta[s, : len(rows)] = dbits[rows]
                sidx16v[s, : len(rows)] = nodes_m[rows].astype(np.int16)
            NSB = NS // PART
            core_in["sdeltat"] = np.ascontiguousarray(
                sdelta.reshape(W, NSB, PART, D)
                .transpose(2, 0, 1, 3)
                .reshape(PART, W * NSB, D)
            )
            s16 = (
                sidx16v.reshape(W, NS // 16, 16)
                .transpose(2, 0, 1)
                .reshape(16, W * (NS // 16))
            )
            core_in["sidx16"] = np.ascontiguousarray(np.tile(s16, (PART // 16, 1)))
        else:
            # scatter targets: 1KB-row index (local*W + slot); pads -> garbage
            sflat = np.empty(Bpad, np.int32)
            sflat[:bm] = (local[sel] * W + slot[sel]).astype(np.int32)
            sflat[bm:] = R * W + (np.arange(Bpad - bm) % (GARBAGE_NODES * W))
            epad = np.zeros((Bpad, D), np.float32)
            epad[:bm] = emb[sel]
            core_in["embt"] = np.ascontiguousarray(
                epad.reshape(T, PART, D).transpose(1, 0, 2).reshape(PART, T * D)
            )
            core_in["sidx"] = np.ascontiguousarray(sflat.reshape(T, PART).T)

        per_core.append(core_in)

    return per_core, sels, T, node_rows, counts, slot, NS


LAST_RESULT = None  # BassKernelResults of the most recent run (for test harness)


def kernel(bank, times, emb, t, ptr, idx):
    import os

    from concourse.bass_utils import run_bass_kernel_spmd

    global LAST_RESULT
    trace = bool(os.environ.get("KERNEL_TRACE"))

    bank = np.ascontiguousarray(np.asarray(bank))
    times = np.ascontiguousarray(np.asarray(times))
    emb = np.ascontiguousarray(np.asarray(emb, dtype=np.float32))
    t = np.asarray(t)
    ptr = np.asarray(ptr)
    idx = np.asarray(idx)

    N, W, D = bank.shape
    B = idx.shape[0]

    per_core, sels, T, node_rows, counts, slot, NS = _host_prep(
        bank, emb, ptr, idx, M, scatter_mode=SCATTER_MODE
    )

    nc = _build_program(node_rows, T, W, D, scatter_mode=SCATTER_MODE, NS=NS)
    try:
        LAST_RESULT = run_bass_kernel_spmd(nc, per_core, list(range(M)), trace=trace)
    except Exception:
        # one retry: the tunneled device occasionally drops a run
        LAST_RESULT = run_bass_kernel_spmd(nc, per_core, list(range(M)), trace=trace)
    res = LAST_RESULT.results

    out_bank = np.empty((B, W, D), np.float32)
    for m in range(M):
        sel = sels[m]
        if len(sel):
            out_bank[sel] = res[m]["outb"][: len(sel)].reshape(-1, W, D)

    # small outputs on host
    idx64 = idx.astype(np.int64)
    times2 = times.copy()
    times2[idx64, slot] = t
    out_times = times2[idx64]
    ptr2 = ptr + counts

    return out_bank, out_times, ptr2
